# revision 1
# baseline (speedup 1.0000x reference)
"""Contextual-attention kernel for Trainium2, 8 NeuronCores, SPMD.

Decomposition (validated against the jax reference in numpy):
  scores[l,p] = rn[l] * sum_kk V[kk,l] * Gbox[kk,p]      (matmul1, kk=9*128)
  E = exp(scores - max_l scores)                          (softmax numerator)
  Mz[p,:] = sum_l E[l,p] * [rn[l]*V_lkk[l,:1152], 1]      (matmul2, Z in last col)
  out = col2im(Mz[:, :1152]/Z) * m/9 + fg*(1-m)           (host)

Sharding: core c handles sample c//2, pixel half c%2 (2048 of 4096 pixels).
No collectives; host scatters inputs / gathers outputs.
"""
import sys
for _p in ('/opt/trn_rl_repo',):
    if _p not in sys.path:
        sys.path.insert(0, _p)

import numpy as np

import concourse.bass as bass
import concourse.mybir as mybir
import concourse.tile as tile
from concourse import bacc
from concourse.bass_isa import ReduceOp
from concourse.bass_utils import run_bass_kernel_spmd

EPS = 1e-7
C, H, W = 128, 64, 64
L = H * W                      # 4096
KK = 9 * C                     # 1152
NC_COUNT = 8
HALF = L // 2                  # 2048 pixels per core
NCHUNK = 4                     # p-chunks of 512 per core
CW = 512                       # chunk width (pixels)
LT = 32                        # l-tiles of 128
PT_PER_CORE = 16               # p-tiles of 128 per core
DT_MM = mybir.dt.float32  # exact; float32r needs producer-side rounding
F32 = mybir.dt.float32

_compiled = None


def _build_program():
    nc = bacc.Bacc("TRN2", target_bir_lowering=False, debug=False)
    vslab_d = nc.dram_tensor("vslab", [C, 3 * 66 * 64], F32, kind="ExternalInput").ap()
    rnt_d = nc.dram_tensor("rnt", [C, LT], F32, kind="ExternalInput").ap()
    gsh_d = nc.dram_tensor("gsh", [9, C, HALF], F32, kind="ExternalInput").ap()
    vlkk2_d = nc.dram_tensor("vlkk2", [LT, C, KK + 1], F32, kind="ExternalInput").ap()
    mout_d = nc.dram_tensor("mout", [PT_PER_CORE, C, KK + 1], F32,
                            kind="ExternalOutput").ap()
    ident_d = nc.dram_tensor("ident", [C, C], F32, kind="ExternalInput").ap()
    ones1_d = nc.dram_tensor("ones1", [1, C], F32, kind="ExternalInput").ap()

    with tile.TileContext(nc) as tc:
        with (
            tc.tile_pool(name="const", bufs=1) as cpool,
            tc.tile_pool(name="gpool", bufs=2) as gpool,
            tc.tile_pool(name="sspool", bufs=1) as sspool,
            tc.tile_pool(name="small", bufs=2) as small,
            tc.tile_pool(name="vbufs", bufs=4) as vpool,
            tc.tile_pool(name="mo", bufs=4) as mopool,
            tc.tile_pool(name="ps1", bufs=2, space="PSUM") as ps1,
            tc.tile_pool(name="psm", bufs=2, space="PSUM") as psm,
            tc.tile_pool(name="ps2", bufs=4, space="PSUM") as ps2,
        ):
            vs = cpool.tile([C, 3 * 66 * 64], F32)
            nc.sync.dma_start(out=vs[:], in_=vslab_d[:])
            rnt = cpool.tile([C, LT], F32)
            nc.sync.dma_start(out=rnt[:], in_=rnt_d[:])
            ident = cpool.tile([C, C], F32)
            nc.sync.dma_start(out=ident[:], in_=ident_d[:])
            ones1 = cpool.tile([1, C], F32)
            nc.sync.dma_start(out=ones1[:], in_=ones1_d[:])

            for ch in range(NCHUNK):
                # ---- load G chunk: [128, 9, 512]
                gt = gpool.tile([C, 9, CW], F32, tag="gt")
                for k in range(9):
                    nc.sync.dma_start(out=gt[:, k, :],
                                      in_=gsh_d[k, :, ch * CW:(ch + 1) * CW])

                # ---- matmul1: ss[l, p] for all 32 l-tiles of this chunk
                ss = sspool.tile([C, LT * CW], F32, tag="ss")
                for lt in range(LT):
                    ps = ps1.tile([C, CW], F32, tag="ps1")
                    for k in range(9):
                        di, dj = k // 3, k % 3
                        base = (dj * 66 + 2 * lt + di) * 64
                        lhsT = vs[:, base:base + 128]
                        nc.tensor.matmul(ps[:], lhsT.bitcast(DT_MM),
                                         gt[:, k, :].bitcast(DT_MM),
                                         start=(k == 0), stop=(k == 8))
                    # drain with per-partition rn scale
                    nc.vector.tensor_scalar(
                        out=ss[:, lt * CW:(lt + 1) * CW], in0=ps[:],
                        scalar1=rnt[:, lt:lt + 1], scalar2=None,
                        op0=mybir.AluOpType.mult)

                # ---- max over l (32 tiles then across partitions)
                mrun = small.tile([C, CW], F32, tag="mrun")
                nc.vector.tensor_copy(out=mrun[:], in_=ss[:, 0:CW])
                for lt in range(1, LT):
                    nc.vector.tensor_tensor(out=mrun[:], in0=mrun[:],
                                            in1=ss[:, lt * CW:(lt + 1) * CW],
                                            op=mybir.AluOpType.max)
                # cross-partition max via PE: per 128-px block, transpose,
                # free-axis max, transpose back, ones-broadcast to all partitions
                mb = small.tile([C, CW], F32, tag="mb", name=f"mb_{ch}")
                for b in range(4):
                    tps = psm.tile([C, C], F32, tag="tp", name=f"tp_{ch}_{b}")
                    nc.tensor.transpose(tps[:], mrun[:, b * C:(b + 1) * C], ident[:])
                    tms = small.tile([C, C], F32, tag="tms", name=f"tms_{ch}_{b}")
                    nc.vector.tensor_copy(out=tms[:], in_=tps[:])
                    mcol = small.tile([C, 1], F32, tag="mcol", name=f"mc_{ch}_{b}")
                    nc.vector.tensor_reduce(mcol[:], tms[:],
                                            axis=mybir.AxisListType.XYZW,
                                            op=mybir.AluOpType.max)
                    tp2 = psm.tile([1, C], F32, tag="tp", name=f"tp2_{ch}_{b}")
                    nc.tensor.transpose(tp2[:], mcol[:], ident[:])
                    mrow = small.tile([1, C], F32, tag="mrow", name=f"mr_{ch}_{b}")
                    nc.vector.tensor_copy(out=mrow[:], in_=tp2[:])
                    bps = psm.tile([C, C], F32, tag="tp", name=f"bp_{ch}_{b}")
                    nc.tensor.matmul(bps[:], ones1[:], mrow[:], start=True, stop=True)
                    nc.vector.tensor_copy(out=mb[:, b * C:(b + 1) * C], in_=bps[:])
                mrun = mb

                # ---- exp(ss - m)
                for lt in range(LT):
                    sl = ss[:, lt * CW:(lt + 1) * CW]
                    nc.vector.tensor_tensor(out=sl, in0=sl, in1=mrun[:],
                                            op=mybir.AluOpType.subtract)
                    nc.scalar.activation(sl, sl, mybir.ActivationFunctionType.Exp)

                # ---- matmul2: Mz[p, kk] = sum_l E[l,p] * vlkk2[l,kk]
                for (c0, c1) in ((0, 512), (512, 1024), (1024, KK + 1)):
                    cw = c1 - c0
                    pss = [ps2.tile([C, 512], F32, tag="ps2", name=f"ps2_{ch}_{c0}_{i}")
                           for i in range(4)]
                    for ls in range(LT):
                        vb = vpool.tile([C, 512], F32, tag="vb")
                        nc.sync.dma_start(out=vb[:, :cw], in_=vlkk2_d[ls, :, c0:c1])
                        for pt in range(4):
                            lhsT = ss[:, ls * CW + pt * 128: ls * CW + (pt + 1) * 128]
                            nc.tensor.matmul(pss[pt][:, :cw], lhsT.bitcast(DT_MM),
                                             vb[:, :cw].bitcast(DT_MM),
                                             start=(ls == 0), stop=(ls == LT - 1))
                    for pt in range(4):
                        mo = mopool.tile([C, 512], F32, tag="mo")
                        nc.vector.tensor_copy(out=mo[:, :cw], in_=pss[pt][:, :cw])
                        nc.sync.dma_start(out=mout_d[ch * 4 + pt, :, c0:c1],
                                          in_=mo[:, :cw])
    nc.compile()
    return nc


def _host_prep(fg, m):
    """Per-sample operand tensors. fg [C,H,W] f32, m [1,H,W] f32."""
    bg = fg * (1.0 - m)
    vslab = (np.pad(bg, ((0, 0), (1, 1), (1, 1))) + EPS).astype(np.float32)

    v_lkk = np.empty((L, KK + 1), np.float32)
    for di in range(3):
        for dj in range(3):
            v_lkk[:, (di * 3 + dj) * C:(di * 3 + dj + 1) * C] = \
                vslab[:, di:di + H, dj:dj + W].reshape(C, L).T
    v_lkk[:, KK] = 1.0

    norm2 = np.sum(v_lkk[:, :KK].astype(np.float64) ** 2, axis=1)
    rn = (1.0 / np.sqrt(norm2)).astype(np.float32)
    rnt = np.ascontiguousarray(rn.reshape(LT, C).T)          # [128, 32]

    v_lkk2 = v_lkk.copy()
    v_lkk2[:, :KK] *= rn[:, None]
    vlkk2 = np.ascontiguousarray(v_lkk2.reshape(LT, C, KK + 1))

    fgpad = np.pad(fg, ((0, 0), (1, 1), (1, 1)))
    G = np.empty((9, C, L), np.float32)
    for di in range(3):
        for dj in range(3):
            Z = np.zeros((C, H + 2, W + 2), np.float32)
            Z[:, 1:H + 1, 1:W + 1] = fgpad[:, di:di + H, dj:dj + W]
            B = sum(Z[:, a:a + H, b:b + W] for a in range(3) for b in range(3))
            G[di * 3 + dj] = B.reshape(C, L)
    return vslab, rnt, vlkk2, G


def _host_post(Mpatch, fg, m):
    """col2im + final combine for one sample. Mpatch [L, 1152]."""
    rec = np.zeros((C, H, W), np.float32)
    Mp = Mpatch.reshape(H, W, 9, C)
    for di in range(3):
        for dj in range(3):
            oy, ox = 1 - di, 1 - dj
            ys, ye = max(0, -oy), min(H, H - oy)
            xs, xe = max(0, -ox), min(W, W - ox)
            rec[:, ys:ye, xs:xe] += np.transpose(
                Mp[ys + oy:ye + oy, xs + ox:xe + ox, di * 3 + dj, :], (2, 0, 1))
    return rec * m / 9.0 + fg * (1.0 - m)


def kernel(foreground, mask, _results_hook=None):
    global _compiled
    foreground = np.asarray(foreground, np.float32)
    mask = np.asarray(mask, np.float32)
    B = foreground.shape[0]

    if _compiled is None:
        _compiled = _build_program()
    nc = _compiled

    in_maps = []
    preps = []
    for s in range(B):
        vslab, rnt, vlkk2, G = _host_prep(foreground[s], mask[s])
        # [C,66,66] -> [C, 3(dj), 66, 64]: vs2[c,dj,y,x] = vslab[c,y,x+dj]
        vslab = np.ascontiguousarray(
            np.stack([vslab[:, :, dj:dj + 64] for dj in range(3)], axis=1)
        ).reshape(C, 3 * 66 * 64)
        preps.append((vslab, rnt, vlkk2, G))
    for core in range(NC_COUNT):
        s, h = core // 2, core % 2
        vslab, rnt, vlkk2, G = preps[s]
        in_maps.append({
            "vslab": vslab,
            "rnt": rnt,
            "gsh": np.ascontiguousarray(G[:, :, h * HALF:(h + 1) * HALF]),
            "vlkk2": vlkk2,
            "ident": np.eye(C, dtype=np.float32),
            "ones1": np.ones((1, C), np.float32),
        })

    res = run_bass_kernel_spmd(nc, in_maps, list(range(NC_COUNT)))
    if _results_hook is not None:
        _results_hook(res)

    out = np.empty_like(foreground)
    for s in range(B):
        halves = []
        for h in range(2):
            mo = np.asarray(res.results[2 * s + h]["mout"])      # [16,128,1153]
            halves.append(mo.transpose(0, 1, 2).reshape(HALF, KK + 1))
        Mz = np.concatenate(halves, axis=0)                       # [L, 1153]
        Mpatch = Mz[:, :KK] / Mz[:, KK:KK + 1]
        out[s] = _host_post(Mpatch, foreground[s], mask[s])
    return out



# revision 7
# speedup vs baseline: 7.4133x; 7.4133x over previous
"""Contextual-attention kernel for Trainium2, 8 NeuronCores, SPMD.

Math (validated in numpy against the jax reference):
  scores[l,p] = rn[l] * sum_kk V[kk,l] * Gbox[kk,p]      (matmul1, kk=9*128)
  A = softmax_l(scores)                                   (exact max, exp, 1/Z)
  Pk[c,p]     = sum_l (rn[l]*V[(k,c),l]) * A[l,p]         (matmul2, per k)
  rec         = col2im(Pk)  ; out = rec*m/9 + fg*(1-m)

All prep (patch slabs, box sums, softmax, col2im, final combine) runs on
device; the host only uploads fg/mask (+tiny norm & boundary-correction
tensors) and downloads the finished half-image per core. Core c handles
sample c//2, row-half c%2. Both halves run the SAME program: every
half-dependent quantity is folded into host-prepared inputs.
"""
import sys
for _p in ('/opt/trn_rl_repo',):
    if _p not in sys.path:
        sys.path.insert(0, _p)

import numpy as np
import ml_dtypes

import concourse.bass as bass
import concourse.mybir as mybir
import concourse.tile as tile
from concourse import bacc
from concourse.bass_utils import run_bass_kernel_spmd

EPS = 1e-7
C, H, W = 128, 64, 64
L = H * W                      # 4096
NC_COUNT = 8
HALF = L // 2                  # 2048 px per core
LT = 32                        # l-tiles of 128
NCH = 8                        # px chunks per core
PC = HALF // NCH               # 256 px per chunk = 4 rows
ROWS = PC // W                 # 4
F32 = mybir.dt.float32
BF = mybir.dt.bfloat16
NPBF = ml_dtypes.bfloat16
AL = mybir.AluOpType

_compiled = None


def _build_program():
    nc = bacc.Bacc("TRN2", target_bir_lowering=False, debug=False)
    fgb_d = nc.dram_tensor("fgb", [C, H, W], BF, kind="ExternalInput").ap()
    fgwin_d = nc.dram_tensor("fgwin", [C, 36, 66], BF, kind="ExternalInput").ap()
    fgh_d = nc.dram_tensor("fgh", [C, 32, W], BF, kind="ExternalInput").ap()
    mk_d = nc.dram_tensor("mk", [1, H, W], F32, kind="ExternalInput").ap()
    mkh_d = nc.dram_tensor("mkh", [1, 32, W], F32, kind="ExternalInput").ap()
    rnt_d = nc.dram_tensor("rnt", [C, LT], F32, kind="ExternalInput").ap()
    rcorr_d = nc.dram_tensor("rcorr", [C, 2, 3, W], BF, kind="ExternalInput").ap()
    ccorr_d = nc.dram_tensor("ccorr", [C, 2, 3, 32], BF, kind="ExternalInput").ap()
    ident_d = nc.dram_tensor("ident", [C, C], BF, kind="ExternalInput").ap()
    mout_d = nc.dram_tensor("mout", [C, 32, W], BF, kind="ExternalOutput").ap()
    bnd_d = nc.dram_tensor("bnd", [C, 2, W], F32, kind="ExternalOutput").ap()

    with tile.TileContext(nc) as tc:
        with (
            tc.tile_pool(name="persist", bufs=1) as pp,
            tc.tile_pool(name="pps", bufs=2, space="PSUM") as pps,
            tc.tile_pool(name="ps1", bufs=2, space="PSUM") as ps1,
            tc.tile_pool(name="ps2", bufs=1, space="PSUM") as ps2,
        ):
            vs = pp.tile([C, 3, 66, 64], BF)
            gt = pp.tile([C, 9, 32, 64], BF)
            vbk = pp.tile([C, LT, 9, C], BF)
            rnt = pp.tile([C, LT], F32)
            identb = pp.tile([C, C], BF)
            ones1f = pp.tile([1, C], F32)
            ones1b = pp.tile([1, C], BF)
            onesPf = pp.tile([C, 1], F32)
            rec = pp.tile([C, 32, W], F32)
            bnd = pp.tile([C, 2, W], F32)
            nc.sync.dma_start(out=rnt[:], in_=rnt_d[:])
            nc.sync.dma_start(out=identb[:], in_=ident_d[:])
            nc.vector.memset(ones1f[:], 1.0)
            nc.vector.memset(ones1b[:], 1.0)
            nc.vector.memset(onesPf[:], 1.0)
            nc.vector.memset(rec[:], 0.0)
            nc.vector.memset(bnd[:], 0.0)

            # ---------------- prep 1: vs (bg patch slabs) ----------------
            with tc.tile_pool(name="prep1", bufs=1) as p1:
                fgbT = p1.tile([C, H, W], BF)
                mkS = p1.tile([1, H, W], F32)
                mb = p1.tile([C, H, W], F32)
                bg = p1.tile([C, H, W], BF)
                nc.sync.dma_start(out=fgbT[:], in_=fgb_d[:])
                nc.sync.dma_start(out=mkS[:], in_=mk_d[:])
                for j in range(8):
                    pb = pps.tile([C, 8, W], F32, tag="pb", name=f"pb_mb{j}")
                    nc.tensor.matmul(pb[:], ones1f[:], mkS[:, j * 8:(j + 1) * 8, :],
                                     start=True, stop=True)
                    nc.vector.tensor_copy(out=mb[:, j * 8:(j + 1) * 8, :], in_=pb[:])
                # mb <- 1 - mb ; bg = fg * (1 - m)
                nc.vector.tensor_scalar(out=mb[:], in0=mb[:], scalar1=-1.0,
                                        scalar2=1.0, op0=AL.mult, op1=AL.add)
                nc.vector.tensor_tensor(out=bg[:], in0=fgbT[:], in1=mb[:],
                                        op=AL.mult)
                # vs[c,dj,y,x] = pad(bg)[c, y, x+dj] + EPS
                nc.vector.memset(vs[:], EPS)
                for dj in range(3):
                    xs, xe = max(0, 1 - dj), min(64, 65 - dj)
                    bxs = xs + dj - 1
                    nc.vector.tensor_scalar(
                        out=vs[:, dj, 1:65, xs:xe],
                        in0=bg[:, :, bxs:bxs + xe - xs],
                        scalar1=EPS, scalar2=None, op0=AL.add)

            # ---------------- prep 2: gt (box-summed G windows) ----------------
            with tc.tile_pool(name="prep2", bufs=1) as p2:
                fgw = p2.tile([C, 36, 66], BF)
                rw = p2.tile([C, 36, 66], F32)
                b1 = p2.tile([C, 34, 66], F32)
                bxf = p2.tile([C, 34, 66], BF)
                rcorr = p2.tile([C, 2, 3, W], BF)
                ccorr = p2.tile([C, 2, 3, 32], BF)
                nc.sync.dma_start(out=fgw[:], in_=fgwin_d[:])
                nc.sync.dma_start(out=rcorr[:], in_=rcorr_d[:])
                nc.sync.dma_start(out=ccorr[:], in_=ccorr_d[:])
                # row box pass (zero-extended via window's own zero cols)
                nc.vector.tensor_tensor(out=rw[:, :, 1:65], in0=fgw[:, :, 0:64],
                                        in1=fgw[:, :, 1:65], op=AL.add)
                nc.vector.tensor_tensor(out=rw[:, :, 1:65], in0=rw[:, :, 1:65],
                                        in1=fgw[:, :, 2:66], op=AL.add)
                nc.vector.tensor_copy(out=rw[:, :, 0], in_=fgw[:, :, 1])
                nc.vector.tensor_copy(out=rw[:, :, 65], in_=fgw[:, :, 64])
                # col box pass
                nc.vector.tensor_tensor(out=b1[:], in0=rw[:, 0:34, :],
                                        in1=rw[:, 1:35, :], op=AL.add)
                nc.vector.tensor_tensor(out=bxf[:], in0=b1[:],
                                        in1=rw[:, 2:36, :], op=AL.add)
                # gt views + corrections
                for di in range(3):
                    for dj in range(3):
                        k = 3 * di + dj
                        nc.vector.tensor_copy(
                            out=gt[:, k], in_=bxf[:, di:di + 32, dj:dj + 64])
                        if dj == 2:
                            nc.vector.tensor_tensor(
                                out=gt[:, k, :, 0], in0=gt[:, k, :, 0],
                                in1=ccorr[:, 0, di, :], op=AL.subtract)
                        if dj == 0:
                            nc.vector.tensor_tensor(
                                out=gt[:, k, :, 63], in0=gt[:, k, :, 63],
                                in1=ccorr[:, 1, di, :], op=AL.subtract)
                        if di == 2:
                            nc.vector.tensor_tensor(
                                out=gt[:, k, 0, :], in0=gt[:, k, 0, :],
                                in1=rcorr[:, 0, dj, :], op=AL.subtract)
                        if di == 0:
                            nc.vector.tensor_tensor(
                                out=gt[:, k, 31, :], in0=gt[:, k, 31, :],
                                in1=rcorr[:, 1, dj, :], op=AL.subtract)

            # ---------------- prep 3: vbk (rn-scaled l-major V) ----------------
            for ls in range(LT):
                for di in range(3):
                    for dj in range(3):
                        k = 3 * di + dj
                        tp = pps.tile([C, C], BF, tag="pb", name=f"tp_{ls}_{k}")
                        nc.tensor.transpose(tp[:], vs[:, dj, 2 * ls + di:2 * ls + di + 2, :],
                                            identb[:])
                        nc.vector.tensor_scalar(
                            out=vbk[:, ls, k, :], in0=tp[:],
                            scalar1=rnt[:, ls:ls + 1], scalar2=None, op0=AL.mult)

            # ---------------- main chunk loop ----------------
            with tc.tile_pool(name="work", bufs=1) as wk:
                ss = wk.tile([C, LT, ROWS, W], F32)
                ee = wk.tile([C, LT, ROWS, W], BF)
                mrun = wk.tile([C, ROWS, W], F32)
                mrunb = wk.tile([C, ROWS, W], BF)
                mx = wk.tile([C, ROWS, W], F32)
                s1b = wk.tile([C, ROWS, W], F32)
                zinv = wk.tile([1, ROWS, W], F32)
                invzb = wk.tile([C, ROWS, W], F32)

                for ch in range(NCH):
                    r0 = ch * ROWS
                    # matmul1: scores for all 32 l-tiles on this px chunk
                    for lt in range(LT):
                        ps = ps1.tile([C, ROWS, W], F32, tag="ps1",
                                      name=f"ps1_{ch}_{lt}")
                        for di in range(3):
                            for dj in range(3):
                                k = 3 * di + dj
                                nc.tensor.matmul(
                                    ps[:], vs[:, dj, 2 * lt + di:2 * lt + di + 2, :],
                                    gt[:, k, r0:r0 + ROWS, :],
                                    start=(k == 0), stop=(k == 8))
                        nc.vector.tensor_scalar(
                            out=ss[:, lt], in0=ps[:], scalar1=rnt[:, lt:lt + 1],
                            scalar2=None, op0=AL.mult)
                    # exact max over l
                    nc.vector.tensor_copy(out=mrun[:], in_=ss[:, 0])
                    for lt in range(1, LT):
                        nc.vector.tensor_tensor(out=mrun[:], in0=mrun[:],
                                                in1=ss[:, lt], op=AL.max)
                    nc.vector.tensor_copy(out=mrunb[:], in_=mrun[:])
                    mrow = wk.tile([1, 2, C], BF, tag="mrow", name=f"mrow_{ch}")
                    for b in range(2):
                        tp = pps.tile([C, C], BF, tag="pb", name=f"mt_{ch}_{b}")
                        nc.tensor.transpose(tp[:], mrunb[:, 2 * b:2 * b + 2, :],
                                            identb[:])
                        mcol = wk.tile([C, 1], F32, tag="mcol",
                                       name=f"mcol_{ch}_{b}")
                        nc.vector.tensor_reduce(mcol[:], tp[:],
                                                axis=mybir.AxisListType.XYZW,
                                                op=AL.max)
                        mcolb = wk.tile([C, 1], BF, tag="mcolb",
                                        name=f"mcolb_{ch}_{b}")
                        nc.vector.tensor_copy(out=mcolb[:], in_=mcol[:])
                        tp2 = pps.tile([1, C], BF, tag="pb", name=f"mt2_{ch}_{b}")
                        nc.tensor.transpose(tp2[:], mcolb[:], identb[:])
                        nc.vector.tensor_copy(out=mrow[:, b, :], in_=tp2[:])
                    pmx = pps.tile([C, ROWS, W], F32, tag="pb", name=f"pmx_{ch}")
                    nc.tensor.matmul(pmx[:], ones1b[:], mrow[:], start=True,
                                     stop=True)
                    nc.vector.tensor_copy(out=mx[:], in_=pmx[:])
                    # exp(ss - mx) -> bf16
                    for lt in range(LT):
                        nc.vector.tensor_tensor(out=ss[:, lt], in0=ss[:, lt],
                                                in1=mx[:], op=AL.subtract)
                        nc.scalar.activation(ee[:, lt], ss[:, lt],
                                             mybir.ActivationFunctionType.Exp)
                    # Z and 1/Z broadcast
                    nc.vector.tensor_tensor(out=s1b[:], in0=ee[:, 0], in1=ee[:, 1],
                                            op=AL.add)
                    for lt in range(2, LT):
                        nc.vector.tensor_tensor(out=s1b[:], in0=s1b[:],
                                                in1=ee[:, lt], op=AL.add)
                    pz = pps.tile([1, ROWS, W], F32, tag="pb", name=f"pz_{ch}")
                    nc.tensor.matmul(pz[:], onesPf[:], s1b[:], start=True, stop=True)
                    nc.vector.reciprocal(zinv[:], pz[:])
                    pzb = pps.tile([C, ROWS, W], F32, tag="pb", name=f"pzb_{ch}")
                    nc.tensor.matmul(pzb[:], ones1f[:], zinv[:], start=True,
                                     stop=True)
                    nc.vector.tensor_copy(out=invzb[:], in_=pzb[:])
                    # A = E / Z (in place, bf16)
                    for lt in range(LT):
                        nc.vector.tensor_tensor(out=ee[:, lt], in0=ee[:, lt],
                                                in1=invzb[:], op=AL.mult)
                    # matmul2 in 3 k-groups of 3, accumulate over l, drain into
                    # col2im shifted adds
                    for g in range(3):
                        pks = []
                        for i in range(3):
                            k = 3 * g + i
                            pk = ps2.tile([C, ROWS, W], F32, tag=f"m2_{i}",
                                          name=f"m2_{ch}_{k}")
                            pks.append(pk)
                        for ls in range(LT):
                            for i in range(3):
                                k = 3 * g + i
                                nc.tensor.matmul(pks[i][:], vbk[:, ls, k, :],
                                                 ee[:, ls],
                                                 start=(ls == 0), stop=(ls == LT - 1))
                        for i in range(3):
                            k = 3 * g + i
                            di, dj = k // 3, k % 3
                            # dest rows d = s + di - 1 (local), with edge drops
                            s0 = 1 if (di == 0 and ch == 0) else 0
                            s1_ = ROWS - 1 if (di == 2 and ch == NCH - 1) else ROWS
                            if s1_ <= s0:
                                continue
                            d0 = r0 + s0 + di - 1
                            xs, xe = max(0, dj - 1), min(64, 63 + dj)
                            sxs = xs + 1 - dj
                            nc.vector.tensor_tensor(
                                out=rec[:, d0:d0 + s1_ - s0, xs:xe],
                                in0=rec[:, d0:d0 + s1_ - s0, xs:xe],
                                in1=pks[i][:, s0:s1_, sxs:sxs + xe - xs],
                                op=AL.add)
                            # boundary rows owed to the neighbour half
                            if di == 2 and ch == NCH - 1:
                                nc.vector.tensor_tensor(
                                    out=bnd[:, 0, xs:xe], in0=bnd[:, 0, xs:xe],
                                    in1=pks[i][:, ROWS - 1, sxs:sxs + xe - xs],
                                    op=AL.add)
                            if di == 0 and ch == 0:
                                nc.vector.tensor_tensor(
                                    out=bnd[:, 1, xs:xe], in0=bnd[:, 1, xs:xe],
                                    in1=pks[i][:, 0, sxs:sxs + xe - xs],
                                    op=AL.add)

                # ---------------- final combine ----------------
                for p in range(4):
                    rr = p * 8
                    fghp = wk.tile([C, 8, W], BF, tag="fghp", name=f"fghp_{p}")
                    nc.sync.dma_start(out=fghp[:], in_=fgh_d[:, rr:rr + 8, :])
                    mkhp = wk.tile([1, 8, W], F32, tag="mkhp", name=f"mkhp_{p}")
                    nc.sync.dma_start(out=mkhp[:], in_=mkh_d[:, rr:rr + 8, :])
                    pmb = pps.tile([C, 8, W], F32, tag="pb", name=f"pmb_{p}")
                    nc.tensor.matmul(pmb[:], ones1f[:], mkhp[:],
                                     start=True, stop=True)
                    tmp = wk.tile([C, 8, W], F32, tag="tmp", name=f"tmp_{p}")
                    nc.vector.tensor_tensor(out=tmp[:], in0=fghp[:], in1=pmb[:],
                                            op=AL.mult)
                    nc.vector.tensor_scalar(
                        out=rec[:, rr:rr + 8, :], in0=rec[:, rr:rr + 8, :],
                        scalar1=1.0 / 9.0, scalar2=None, op0=AL.mult)
                    nc.vector.tensor_tensor(out=rec[:, rr:rr + 8, :],
                                            in0=rec[:, rr:rr + 8, :], in1=pmb[:],
                                            op=AL.mult)
                    nc.vector.tensor_tensor(out=rec[:, rr:rr + 8, :],
                                            in0=rec[:, rr:rr + 8, :], in1=fghp[:],
                                            op=AL.add)
                    ob = wk.tile([C, 8, W], BF, tag="ob", name=f"ob_{p}")
                    nc.vector.tensor_tensor(out=ob[:], in0=rec[:, rr:rr + 8, :],
                                            in1=tmp[:], op=AL.subtract)
                    nc.sync.dma_start(out=mout_d[:, rr:rr + 8, :], in_=ob[:])
                nc.sync.dma_start(out=bnd_d[:], in_=bnd[:])
    nc.compile()
    return nc


def _host_prep_sample(fg, m):
    """Per-sample host tensors. fg [C,H,W] f32, m [1,H,W] f32."""
    bg = fg * (1.0 - m)
    bgp = np.pad(bg, ((0, 0), (1, 1), (1, 1)))
    vs = np.stack([bgp[:, :, dj:dj + 64] for dj in range(3)], 1).astype(np.float32)
    vs += EPS
    sq = vs * vs
    t = sq.sum(axis=1)
    u = t[:, 0:64] + t[:, 1:65] + t[:, 2:66]
    norm2 = u.sum(axis=0).reshape(L)
    rn = 1.0 / np.sqrt(norm2)
    rnt = np.ascontiguousarray(rn.reshape(LT, C).T).astype(np.float32)

    fgE = np.pad(fg, ((0, 0), (2, 2), (2, 2)))          # [C,68,68]
    fgpad = np.pad(fg, ((0, 0), (1, 1), (1, 1)))
    rb1 = fgE[:, 2, 0:66] + fgE[:, 2, 1:67] + fgE[:, 2, 2:68]
    rb64 = fgE[:, 65, 0:66] + fgE[:, 65, 1:67] + fgE[:, 65, 2:68]
    cb1 = fgE[:, 0:66, 2] + fgE[:, 1:67, 2] + fgE[:, 2:68, 2]
    cb64 = fgE[:, 0:66, 65] + fgE[:, 1:67, 65] + fgE[:, 2:68, 65]
    return rnt, fgE, fgpad, rb1, rb64, cb1, cb64


def _host_prep_core(fg, m, h, samp):
    rnt, fgE, fgpad, rb1, rb64, cb1, cb64 = samp
    R0 = 32 * h
    fgwin = fgE[:, R0:R0 + 36, 1:67]
    rcorr = np.zeros((C, 2, 3, W), np.float32)
    if h == 0:
        for dj in range(3):
            rcorr[:, 0, dj, :] = rb1[:, dj:dj + 64]
        rcorr[:, 0, 2, 0] -= fgpad[:, 1, 1]
        rcorr[:, 0, 0, 63] -= fgpad[:, 1, 64]
    else:
        for dj in range(3):
            rcorr[:, 1, dj, :] = rb64[:, dj:dj + 64]
        rcorr[:, 1, 2, 0] -= fgpad[:, 64, 1]
        rcorr[:, 1, 0, 63] -= fgpad[:, 64, 64]
    ccorr = np.zeros((C, 2, 3, 32), np.float32)
    for di in range(3):
        ccorr[:, 0, di, :] = cb1[:, R0 + di:R0 + di + 32]
        ccorr[:, 1, di, :] = cb64[:, R0 + di:R0 + di + 32]
    return {
        "fgb": np.ascontiguousarray(fg).astype(NPBF),
        "fgwin": np.ascontiguousarray(fgwin).astype(NPBF),
        "fgh": np.ascontiguousarray(fg[:, R0:R0 + 32, :]).astype(NPBF),
        "mk": np.ascontiguousarray(m).astype(np.float32),
        "mkh": np.ascontiguousarray(m[:, R0:R0 + 32, :]).astype(np.float32),
        "rnt": rnt,
        "rcorr": rcorr.astype(NPBF),
        "ccorr": ccorr.astype(NPBF),
        "ident": np.eye(C, dtype=NPBF),
    }


def kernel(foreground, mask, _results_hook=None):
    global _compiled
    foreground = np.asarray(foreground, np.float32)
    mask = np.asarray(mask, np.float32)
    B = foreground.shape[0]

    if _compiled is None:
        _compiled = _build_program()
    nc = _compiled

    in_maps = []
    samps = [_host_prep_sample(foreground[s], mask[s]) for s in range(B)]
    for core in range(NC_COUNT):
        s, h = core // 2, core % 2
        in_maps.append(_host_prep_core(foreground[s], mask[s], h, samps[s]))

    res = run_bass_kernel_spmd(nc, in_maps, list(range(NC_COUNT)))
    if _results_hook is not None:
        _results_hook(res)

    out = np.empty_like(foreground)
    for s in range(B):
        m0 = np.asarray(res.results[2 * s]["mout"]).astype(np.float32)
        m1 = np.asarray(res.results[2 * s + 1]["mout"]).astype(np.float32)
        b0 = np.asarray(res.results[2 * s]["bnd"]).astype(np.float32)
        b1 = np.asarray(res.results[2 * s + 1]["bnd"]).astype(np.float32)
        out[s, :, 0:32] = m0
        out[s, :, 32:64] = m1
        out[s, :, 31] += b1[:, 1, :] * mask[s, 0, 31] / 9.0
        out[s, :, 32] += b0[:, 0, :] * mask[s, 0, 32] / 9.0
    return out


# revision 9
# speedup vs baseline: 9.5655x; 1.2903x over previous
"""Contextual-attention kernel for Trainium2, 8 NeuronCores, SPMD.

Math (validated in numpy against the jax reference):
  scores[l,p] = rn[l] * sum_kk V[kk,l] * Gbox[kk,p]      (matmul1, kk=9*128)
  A = softmax_l(scores)                                   (exact max, exp, 1/Z)
  Pk[c,p]     = sum_l (rn[l]*V[(k,c),l]) * A[l,p]         (matmul2, per k)
  rec         = col2im(Pk)  ; out = rec*m/9 + fg*(1-m)

All prep (patch slabs, box sums, softmax, col2im, final combine) runs on
device; the host only uploads fg/mask (+tiny norm & boundary-correction
tensors) and downloads the finished half-image per core. Core c handles
sample c//2, row-half c%2. Both halves run the SAME program: every
half-dependent quantity is folded into host-prepared inputs.
"""
import sys
for _p in ('/opt/trn_rl_repo',):
    if _p not in sys.path:
        sys.path.insert(0, _p)

import numpy as np
import ml_dtypes

import concourse.bass as bass
import concourse.mybir as mybir
import concourse.tile as tile
from concourse import bacc
from concourse.bass_utils import run_bass_kernel_spmd

EPS = 1e-7
C, H, W = 128, 64, 64
L = H * W                      # 4096
NC_COUNT = 8
HALF = L // 2                  # 2048 px per core
LT = 32                        # l-tiles of 128
NCH = 8                        # px chunks per core
PC = HALF // NCH               # 256 px per chunk = 4 rows
ROWS = PC // W                 # 4
F32 = mybir.dt.float32
BF = mybir.dt.bfloat16
NPBF = ml_dtypes.bfloat16
AL = mybir.AluOpType

OFG = 0
OWIN = 4096
ORC = OWIN + 2376
OCC = ORC + 384
ORN = OCC + 192
NB = ORN + 32

_compiled = None


def _build_program():
    nc = bacc.Bacc("TRN2", target_bir_lowering=False, debug=False)
    binp_d = nc.dram_tensor("binp", [C, NB], BF, kind="ExternalInput").ap()
    minp_d = nc.dram_tensor("minp", [1, 6144], F32, kind="ExternalInput").ap()
    mout_d = nc.dram_tensor("mout", [C, 34, W], BF, kind="ExternalOutput").ap()
    fgb_d = binp_d[:, OFG:OFG + 4096].rearrange("p (a b) -> p a b", a=H)
    fgwin_d = binp_d[:, OWIN:OWIN + 2376].rearrange("p (a b) -> p a b", a=36)
    rcorr_d = binp_d[:, ORC:ORC + 384].rearrange("p (a b c) -> p a b c", a=2, b=3)
    ccorr_d = binp_d[:, OCC:OCC + 192].rearrange("p (a b c) -> p a b c", a=2, b=3)
    rnt_d = binp_d[:, ORN:ORN + LT]
    mk_d = minp_d[:, 0:4096].rearrange("p (a b) -> p a b", a=H)
    mkh_d = minp_d[:, 4096:6144].rearrange("p (a b) -> p a b", a=32)

    with tile.TileContext(nc) as tc:
        with (
            tc.tile_pool(name="persist", bufs=1) as pp,
            tc.tile_pool(name="pps", bufs=2, space="PSUM") as pps,
            tc.tile_pool(name="ps1", bufs=2, space="PSUM") as ps1,
            tc.tile_pool(name="ps2", bufs=1, space="PSUM") as ps2,
        ):
            vs = pp.tile([C, 3, 66, 64], BF)
            gt = pp.tile([C, 9, 32, 64], BF)
            vbk = pp.tile([C, LT, 9, C], BF)
            rntb = pp.tile([C, LT], BF)
            rnt = pp.tile([C, LT], F32)
            identb = pp.tile([C, C], BF)
            ones1f = pp.tile([1, C], F32)
            ones1b = pp.tile([1, C], BF)
            onesPf = pp.tile([C, 1], F32)
            rec = pp.tile([C, 32, W], F32)
            bnd = pp.tile([C, 2, W], F32)
            nc.sync.dma_start(out=rntb[:], in_=rnt_d)
            nc.vector.tensor_copy(out=rnt[:], in_=rntb[:])
            nc.vector.memset(identb[:], 1.0)
            nc.gpsimd.affine_select(
                out=identb[:], in_=identb[:], pattern=[[-1, C]],
                compare_op=AL.is_equal, fill=0.0, base=0, channel_multiplier=1)
            nc.vector.memset(ones1f[:], 1.0)
            nc.vector.memset(ones1b[:], 1.0)
            nc.vector.memset(onesPf[:], 1.0)
            nc.vector.memset(rec[:], 0.0)
            nc.vector.memset(bnd[:], 0.0)

            # ---------------- prep 1: vs (bg patch slabs) ----------------
            with tc.tile_pool(name="prep1", bufs=1) as p1:
                fgbT = p1.tile([C, H, W], BF)
                mkS = p1.tile([1, H, W], F32)
                mb = p1.tile([C, H, W], F32)
                bg = p1.tile([C, H, W], BF)
                nc.sync.dma_start(out=fgbT[:], in_=fgb_d)
                nc.sync.dma_start(out=mkS[:], in_=mk_d)
                for j in range(8):
                    pb = pps.tile([C, 8, W], F32, tag="pb", name=f"pb_mb{j}")
                    nc.tensor.matmul(pb[:], ones1f[:], mkS[:, j * 8:(j + 1) * 8, :],
                                     start=True, stop=True)
                    nc.vector.tensor_copy(out=mb[:, j * 8:(j + 1) * 8, :], in_=pb[:])
                # mb <- 1 - mb ; bg = fg * (1 - m)
                nc.vector.tensor_scalar(out=mb[:], in0=mb[:], scalar1=-1.0,
                                        scalar2=1.0, op0=AL.mult, op1=AL.add)
                nc.vector.tensor_tensor(out=bg[:], in0=fgbT[:], in1=mb[:],
                                        op=AL.mult)
                # vs[c,dj,y,x] = pad(bg)[c, y, x+dj] + EPS
                nc.vector.memset(vs[:], EPS)
                for dj in range(3):
                    xs, xe = max(0, 1 - dj), min(64, 65 - dj)
                    bxs = xs + dj - 1
                    nc.vector.tensor_scalar(
                        out=vs[:, dj, 1:65, xs:xe],
                        in0=bg[:, :, bxs:bxs + xe - xs],
                        scalar1=EPS, scalar2=None, op0=AL.add)

            # ---------------- prep 2: gt (box-summed G windows) ----------------
            with tc.tile_pool(name="prep2", bufs=1) as p2:
                fgw = p2.tile([C, 36, 66], BF)
                rw = p2.tile([C, 36, 66], F32)
                b1 = p2.tile([C, 34, 66], F32)
                bxf = p2.tile([C, 34, 66], BF)
                rcorr = p2.tile([C, 2, 3, W], BF)
                ccorr = p2.tile([C, 2, 3, 32], BF)
                nc.sync.dma_start(out=fgw[:], in_=fgwin_d)
                nc.sync.dma_start(out=rcorr[:], in_=rcorr_d)
                nc.sync.dma_start(out=ccorr[:], in_=ccorr_d)
                # row box pass (zero-extended via window's own zero cols)
                nc.vector.tensor_tensor(out=rw[:, :, 1:65], in0=fgw[:, :, 0:64],
                                        in1=fgw[:, :, 1:65], op=AL.add)
                nc.vector.tensor_tensor(out=rw[:, :, 1:65], in0=rw[:, :, 1:65],
                                        in1=fgw[:, :, 2:66], op=AL.add)
                nc.vector.tensor_copy(out=rw[:, :, 0], in_=fgw[:, :, 1])
                nc.vector.tensor_copy(out=rw[:, :, 65], in_=fgw[:, :, 64])
                # col box pass
                nc.vector.tensor_tensor(out=b1[:], in0=rw[:, 0:34, :],
                                        in1=rw[:, 1:35, :], op=AL.add)
                nc.vector.tensor_tensor(out=bxf[:], in0=b1[:],
                                        in1=rw[:, 2:36, :], op=AL.add)
                # gt views + corrections
                for di in range(3):
                    for dj in range(3):
                        k = 3 * di + dj
                        nc.vector.tensor_copy(
                            out=gt[:, k], in_=bxf[:, di:di + 32, dj:dj + 64])
                        if dj == 2:
                            nc.vector.tensor_tensor(
                                out=gt[:, k, :, 0], in0=gt[:, k, :, 0],
                                in1=ccorr[:, 0, di, :], op=AL.subtract)
                        if dj == 0:
                            nc.vector.tensor_tensor(
                                out=gt[:, k, :, 63], in0=gt[:, k, :, 63],
                                in1=ccorr[:, 1, di, :], op=AL.subtract)
                        if di == 2:
                            nc.vector.tensor_tensor(
                                out=gt[:, k, 0, :], in0=gt[:, k, 0, :],
                                in1=rcorr[:, 0, dj, :], op=AL.subtract)
                        if di == 0:
                            nc.vector.tensor_tensor(
                                out=gt[:, k, 31, :], in0=gt[:, k, 31, :],
                                in1=rcorr[:, 1, dj, :], op=AL.subtract)

            # ---------------- prep 3: vbk (rn-scaled l-major V) ----------------
            for ls in range(LT):
                for di in range(3):
                    for dj in range(3):
                        k = 3 * di + dj
                        tp = pps.tile([C, C], BF, tag="pb", name=f"tp_{ls}_{k}")
                        nc.tensor.transpose(tp[:], vs[:, dj, 2 * ls + di:2 * ls + di + 2, :],
                                            identb[:])
                        nc.vector.tensor_scalar(
                            out=vbk[:, ls, k, :], in0=tp[:],
                            scalar1=rnt[:, ls:ls + 1], scalar2=None, op0=AL.mult)

            # ---------------- main chunk loop ----------------
            with tc.tile_pool(name="work", bufs=1) as wk:
                ss = wk.tile([C, LT, ROWS, W], F32)
                ee = wk.tile([C, LT, ROWS, W], BF)
                mrun = wk.tile([C, ROWS, W], F32)
                mrunb = wk.tile([C, ROWS, W], BF)
                mx = wk.tile([C, ROWS, W], F32)
                s1b = wk.tile([C, ROWS, W], F32)
                zinv = wk.tile([1, ROWS, W], F32)
                invzb = wk.tile([C, ROWS, W], F32)

                for ch in range(NCH):
                    r0 = ch * ROWS
                    # matmul1: scores for all 32 l-tiles on this px chunk
                    for lt in range(LT):
                        ps = ps1.tile([C, ROWS, W], F32, tag="ps1",
                                      name=f"ps1_{ch}_{lt}")
                        for di in range(3):
                            for dj in range(3):
                                k = 3 * di + dj
                                nc.tensor.matmul(
                                    ps[:], vs[:, dj, 2 * lt + di:2 * lt + di + 2, :],
                                    gt[:, k, r0:r0 + ROWS, :],
                                    start=(k == 0), stop=(k == 8))
                        nc.vector.tensor_scalar(
                            out=ss[:, lt], in0=ps[:], scalar1=rnt[:, lt:lt + 1],
                            scalar2=None, op0=AL.mult)
                    # exact max over l
                    nc.vector.tensor_copy(out=mrun[:], in_=ss[:, 0])
                    for lt in range(1, LT):
                        nc.vector.tensor_tensor(out=mrun[:], in0=mrun[:],
                                                in1=ss[:, lt], op=AL.max)
                    nc.vector.tensor_copy(out=mrunb[:], in_=mrun[:])
                    mrow = wk.tile([1, 2, C], BF, tag="mrow", name=f"mrow_{ch}")
                    for b in range(2):
                        tp = pps.tile([C, C], BF, tag="pb", name=f"mt_{ch}_{b}")
                        nc.tensor.transpose(tp[:], mrunb[:, 2 * b:2 * b + 2, :],
                                            identb[:])
                        mcol = wk.tile([C, 1], F32, tag="mcol",
                                       name=f"mcol_{ch}_{b}")
                        nc.vector.tensor_reduce(mcol[:], tp[:],
                                                axis=mybir.AxisListType.XYZW,
                                                op=AL.max)
                        mcolb = wk.tile([C, 1], BF, tag="mcolb",
                                        name=f"mcolb_{ch}_{b}")
                        nc.vector.tensor_copy(out=mcolb[:], in_=mcol[:])
                        tp2 = pps.tile([1, C], BF, tag="pb", name=f"mt2_{ch}_{b}")
                        nc.tensor.transpose(tp2[:], mcolb[:], identb[:])
                        nc.vector.tensor_copy(out=mrow[:, b, :], in_=tp2[:])
                    pmx = pps.tile([C, ROWS, W], F32, tag="pb", name=f"pmx_{ch}")
                    nc.tensor.matmul(pmx[:], ones1b[:], mrow[:], start=True,
                                     stop=True)
                    nc.vector.tensor_copy(out=mx[:], in_=pmx[:])
                    # exp(ss - mx) -> bf16
                    for lt in range(LT):
                        nc.vector.tensor_tensor(out=ss[:, lt], in0=ss[:, lt],
                                                in1=mx[:], op=AL.subtract)
                        nc.scalar.activation(ee[:, lt], ss[:, lt],
                                             mybir.ActivationFunctionType.Exp)
                    # Z and 1/Z broadcast
                    nc.vector.tensor_tensor(out=s1b[:], in0=ee[:, 0], in1=ee[:, 1],
                                            op=AL.add)
                    for lt in range(2, LT):
                        nc.vector.tensor_tensor(out=s1b[:], in0=s1b[:],
                                                in1=ee[:, lt], op=AL.add)
                    pz = pps.tile([1, ROWS, W], F32, tag="pb", name=f"pz_{ch}")
                    nc.tensor.matmul(pz[:], onesPf[:], s1b[:], start=True, stop=True)
                    nc.vector.reciprocal(zinv[:], pz[:])
                    pzb = pps.tile([C, ROWS, W], F32, tag="pb", name=f"pzb_{ch}")
                    nc.tensor.matmul(pzb[:], ones1f[:], zinv[:], start=True,
                                     stop=True)
                    nc.vector.tensor_copy(out=invzb[:], in_=pzb[:])
                    # A = E / Z (in place, bf16)
                    for lt in range(LT):
                        nc.vector.tensor_tensor(out=ee[:, lt], in0=ee[:, lt],
                                                in1=invzb[:], op=AL.mult)
                    # matmul2 in 3 k-groups of 3, accumulate over l, drain into
                    # col2im shifted adds
                    for g in range(3):
                        pks = []
                        for i in range(3):
                            k = 3 * g + i
                            pk = ps2.tile([C, ROWS, W], F32, tag=f"m2_{i}",
                                          name=f"m2_{ch}_{k}")
                            pks.append(pk)
                        for ls in range(LT):
                            for i in range(3):
                                k = 3 * g + i
                                nc.tensor.matmul(pks[i][:], vbk[:, ls, k, :],
                                                 ee[:, ls],
                                                 start=(ls == 0), stop=(ls == LT - 1))
                        for i in range(3):
                            k = 3 * g + i
                            di, dj = k // 3, k % 3
                            # dest rows d = s + di - 1 (local), with edge drops
                            s0 = 1 if (di == 0 and ch == 0) else 0
                            s1_ = ROWS - 1 if (di == 2 and ch == NCH - 1) else ROWS
                            if s1_ <= s0:
                                continue
                            d0 = r0 + s0 + di - 1
                            xs, xe = max(0, dj - 1), min(64, 63 + dj)
                            sxs = xs + 1 - dj
                            nc.vector.tensor_tensor(
                                out=rec[:, d0:d0 + s1_ - s0, xs:xe],
                                in0=rec[:, d0:d0 + s1_ - s0, xs:xe],
                                in1=pks[i][:, s0:s1_, sxs:sxs + xe - xs],
                                op=AL.add)
                            # boundary rows owed to the neighbour half
                            if di == 2 and ch == NCH - 1:
                                nc.vector.tensor_tensor(
                                    out=bnd[:, 0, xs:xe], in0=bnd[:, 0, xs:xe],
                                    in1=pks[i][:, ROWS - 1, sxs:sxs + xe - xs],
                                    op=AL.add)
                            if di == 0 and ch == 0:
                                nc.vector.tensor_tensor(
                                    out=bnd[:, 1, xs:xe], in0=bnd[:, 1, xs:xe],
                                    in1=pks[i][:, 0, sxs:sxs + xe - xs],
                                    op=AL.add)

                # ---------------- final combine ----------------
                for p in range(4):
                    rr = p * 8
                    fghp = wk.tile([C, 8, W], BF, tag="fghp", name=f"fghp_{p}")
                    nc.sync.dma_start(out=fghp[:],
                                      in_=fgwin_d[:, rr + 2:rr + 10, 1:65])
                    mkhp = wk.tile([1, 8, W], F32, tag="mkhp", name=f"mkhp_{p}")
                    nc.sync.dma_start(out=mkhp[:], in_=mkh_d[:, rr:rr + 8, :])
                    pmb = pps.tile([C, 8, W], F32, tag="pb", name=f"pmb_{p}")
                    nc.tensor.matmul(pmb[:], ones1f[:], mkhp[:],
                                     start=True, stop=True)
                    tmp = wk.tile([C, 8, W], F32, tag="tmp", name=f"tmp_{p}")
                    nc.vector.tensor_tensor(out=tmp[:], in0=fghp[:], in1=pmb[:],
                                            op=AL.mult)
                    nc.vector.tensor_scalar(
                        out=rec[:, rr:rr + 8, :], in0=rec[:, rr:rr + 8, :],
                        scalar1=1.0 / 9.0, scalar2=None, op0=AL.mult)
                    nc.vector.tensor_tensor(out=rec[:, rr:rr + 8, :],
                                            in0=rec[:, rr:rr + 8, :], in1=pmb[:],
                                            op=AL.mult)
                    nc.vector.tensor_tensor(out=rec[:, rr:rr + 8, :],
                                            in0=rec[:, rr:rr + 8, :], in1=fghp[:],
                                            op=AL.add)
                    ob = wk.tile([C, 8, W], BF, tag="ob", name=f"ob_{p}")
                    nc.vector.tensor_tensor(out=ob[:], in0=rec[:, rr:rr + 8, :],
                                            in1=tmp[:], op=AL.subtract)
                    nc.sync.dma_start(out=mout_d[:, rr:rr + 8, :], in_=ob[:])
                bndb = wk.tile([C, 2, W], BF)
                nc.vector.tensor_copy(out=bndb[:], in_=bnd[:])
                nc.sync.dma_start(out=mout_d[:, 32:34, :], in_=bndb[:])
    nc.compile()
    return nc


def _host_prep_sample(fg, m):
    """Per-sample host tensors. fg [C,H,W] f32, m [1,H,W] f32."""
    bg = fg * (1.0 - m)
    bgp = np.pad(bg, ((0, 0), (1, 1), (1, 1)))
    vs = np.stack([bgp[:, :, dj:dj + 64] for dj in range(3)], 1).astype(np.float32)
    vs += EPS
    sq = vs * vs
    t = sq.sum(axis=1)
    u = t[:, 0:64] + t[:, 1:65] + t[:, 2:66]
    norm2 = u.sum(axis=0).reshape(L)
    rn = 1.0 / np.sqrt(norm2)
    rnt = np.ascontiguousarray(rn.reshape(LT, C).T).astype(np.float32)

    fgE = np.pad(fg, ((0, 0), (2, 2), (2, 2)))          # [C,68,68]
    fgpad = np.pad(fg, ((0, 0), (1, 1), (1, 1)))
    rb1 = fgE[:, 2, 0:66] + fgE[:, 2, 1:67] + fgE[:, 2, 2:68]
    rb64 = fgE[:, 65, 0:66] + fgE[:, 65, 1:67] + fgE[:, 65, 2:68]
    cb1 = fgE[:, 0:66, 2] + fgE[:, 1:67, 2] + fgE[:, 2:68, 2]
    cb64 = fgE[:, 0:66, 65] + fgE[:, 1:67, 65] + fgE[:, 2:68, 65]
    return rnt, fgE, fgpad, rb1, rb64, cb1, cb64


def _host_prep_core(fg, m, h, samp):
    rnt, fgE, fgpad, rb1, rb64, cb1, cb64 = samp
    R0 = 32 * h
    fgwin = fgE[:, R0:R0 + 36, 1:67]
    rcorr = np.zeros((C, 2, 3, W), np.float32)
    if h == 0:
        for dj in range(3):
            rcorr[:, 0, dj, :] = rb1[:, dj:dj + 64]
        rcorr[:, 0, 2, 0] -= fgpad[:, 1, 1]
        rcorr[:, 0, 0, 63] -= fgpad[:, 1, 64]
    else:
        for dj in range(3):
            rcorr[:, 1, dj, :] = rb64[:, dj:dj + 64]
        rcorr[:, 1, 2, 0] -= fgpad[:, 64, 1]
        rcorr[:, 1, 0, 63] -= fgpad[:, 64, 64]
    ccorr = np.zeros((C, 2, 3, 32), np.float32)
    for di in range(3):
        ccorr[:, 0, di, :] = cb1[:, R0 + di:R0 + di + 32]
        ccorr[:, 1, di, :] = cb64[:, R0 + di:R0 + di + 32]
    binp = np.concatenate([
        fg.reshape(C, 4096), fgwin.reshape(C, 2376),
        rcorr.reshape(C, 384), ccorr.reshape(C, 192), rnt,
    ], axis=1).astype(NPBF)
    minp = np.concatenate([m.reshape(1, 4096),
                           m[:, R0:R0 + 32, :].reshape(1, 2048)],
                          axis=1).astype(np.float32)
    return {"binp": binp, "minp": minp}


def kernel(foreground, mask, _results_hook=None):
    global _compiled
    foreground = np.asarray(foreground, np.float32)
    mask = np.asarray(mask, np.float32)
    B = foreground.shape[0]

    if _compiled is None:
        _compiled = _build_program()
    nc = _compiled

    in_maps = []
    samps = [_host_prep_sample(foreground[s], mask[s]) for s in range(B)]
    for core in range(NC_COUNT):
        s, h = core // 2, core % 2
        in_maps.append(_host_prep_core(foreground[s], mask[s], h, samps[s]))

    res = run_bass_kernel_spmd(nc, in_maps, list(range(NC_COUNT)))
    if _results_hook is not None:
        _results_hook(res)

    out = np.empty_like(foreground)
    for s in range(B):
        m0 = np.asarray(res.results[2 * s]["mout"]).astype(np.float32)
        m1 = np.asarray(res.results[2 * s + 1]["mout"]).astype(np.float32)
        out[s, :, 0:32] = m0[:, 0:32]
        out[s, :, 32:64] = m1[:, 0:32]
        out[s, :, 31] += m1[:, 33] * mask[s, 0, 31] / 9.0
        out[s, :, 32] += m0[:, 32] * mask[s, 0, 32] / 9.0
    return out
